# revision 1
# baseline (speedup 1.0000x reference)
"""Conditional InstanceNorm1D on 8 Trainium2 NeuronCores.

x: [32, 256, 8192] f32. Per-(b, c) instance norm over L (biased var), then a
per-sample style affine: y = x_hat * weight[style_ids[b], c] + bias[style_ids[b], c].

Sharding: pure data parallel over batch. Each core gets 4 samples ->
1024 (b, c) rows of length 8192, processed as 8 tiles of [128 partitions, 8192].
The tiny [S, C] style tables are gathered host-side into per-row scale/shift
columns so the device kernel has no indirect addressing.

Per tile the device does:
  mean/var  : 16x bn_stats (512-elem subgroups) + bn_aggr       (VectorE)
  rstd      : sqrt(var + eps) on ScalarE, reciprocal on VectorE
  fold      : sc = rstd * w_row ; sh = b_row - mean * sc        (VectorE, [128,1])
  apply     : y = Identity(sc * x + sh) in place                (ScalarE)
Loads are issued on the sync sequencer (HWDGE), stores on the scalar
sequencer (HWDGE) so load and store issue never serialize on one queue.
"""

import numpy as np

import concourse.bacc as bacc
import concourse.bass as bass
import concourse.tile as tile
from concourse import mybir
from concourse.bass_utils import run_bass_kernel_spmd

B, C, L, S = 32, 256, 8192, 4
N_CORES = 8
B_PER = B // N_CORES            # 4 samples per core
ROWS = B_PER * C                # 1024 (b, c) rows per core
P = 128                         # SBUF partitions
EPS = 1e-5
F32 = mybir.dt.float32
BN_FMAX = 512                   # bn_stats free-dim hardware limit


def build_nc(rows: int = ROWS, length: int = L, xbufs: int = 4,
             reps: int = 1, loop_reps: int = 0, rpp: int = 1,
             stats_sub: int = 0, alt_queues: bool = False) -> bass.Bass:
    """reps > 1 unrolls the whole pass inside one NEFF; loop_reps > 0 wraps
    the pass in a hardware For_i loop (benchmarking only: the
    (T(R2)-T(R1))/(R2-R1) delta cancels the ~90 ms axon dispatch cost).
    rpp = row-blocks of 128 per SBUF tile: rpp=2 doubles each DMA to 8 MiB
    (partition p holds rows h*128+p for h in 0..rpp, so the [P, ntiles]
    weight/bias table layout is unchanged)."""
    nblocks = rows // P
    ntiles = nblocks // rpp
    nsub = length // BN_FMAX
    # stats_sub < nsub computes stats over a prefix only — WRONG results,
    # benchmarking-only knob to probe whether VectorE binds the interior.
    nsub_used = stats_sub or nsub

    # Bacc (not plain Bass): its finalize() runs generate_event_semaphores,
    # which splits multi-sem waits — TRN2 compute instructions carry at most
    # one sync wait, and walrus rejects the program otherwise.
    nc = bacc.Bacc()
    x_d = nc.dram_tensor("x", [rows, length], F32, kind="ExternalInput")
    w_d = nc.dram_tensor("w", [P, nblocks], F32, kind="ExternalInput")
    b_d = nc.dram_tensor("b", [P, nblocks], F32, kind="ExternalInput")
    y_d = nc.dram_tensor("y", [rows, length], F32, kind="ExternalOutput")

    with tile.TileContext(nc) as tc:
        with (
            tc.tile_pool(name="xp", bufs=xbufs) as xp,
            tc.tile_pool(name="consts", bufs=1) as consts,
            tc.tile_pool(name="stats", bufs=ntiles) as stats,
        ):
            wt_in = consts.tile([P, nblocks], F32)
            bt_in = consts.tile([P, nblocks], F32)
            nc.sync.dma_start(out=wt_in[:], in_=w_d[:])
            nc.sync.dma_start(out=bt_in[:], in_=b_d[:])
            # bounce through a DVE copy: walrus rejects TensorTensor
            # instructions that need a DMA-sem wait (1 wait slot), so make
            # the copy absorb the DMA wait and feed DVE-produced tiles to
            # the per-tile TT ops.
            wt = consts.tile([P, nblocks], F32)
            bt = consts.tile([P, nblocks], F32)
            nc.vector.tensor_copy(wt[:], wt_in[:])
            nc.vector.tensor_copy(bt[:], bt_in[:])
            eps_t = consts.tile([P, 1], F32)
            nc.vector.memset(eps_t[:], EPS)

            def emit_body():
                for i in range(ntiles * reps):
                    i = i % ntiles
                    rows0 = i * P * rpp
                    x_view = x_d[rows0:rows0 + P * rpp, :].rearrange(
                        "(k p) l -> p k l", p=P)
                    y_view = y_d[rows0:rows0 + P * rpp, :].rearrange(
                        "(k p) l -> p k l", p=P)
                    xt = xp.tile([P, rpp, length], F32)
                    # alt_queues: alternate tiles between the two HWDGE
                    # rings (SP / ACT sequencers), stores on the other ring
                    ld = nc.scalar if (alt_queues and i % 2) else nc.sync
                    st_eng = nc.sync if (alt_queues and i % 2) else nc.scalar
                    ld.dma_start(out=xt[:], in_=x_view)

                    st = stats.tile([P, rpp, nsub_used, 6], F32)
                    mv = stats.tile([P, rpp, 2], F32)
                    sc = stats.tile([P, rpp], F32)
                    sh = stats.tile([P, rpp], F32)
                    for h in range(rpp):
                        xr = xt[:, h, :].rearrange("p (n f) -> p n f", f=BN_FMAX)
                        for j in range(nsub_used):
                            nc.vector.bn_stats(out=st[:, h, j, :], in_=xr[:, j, :])
                        nc.vector.bn_aggr(out=mv[:, h, :], in_=st[:, h])
                        blk = i * rpp + h
                        # sc = w_row / sqrt(var + eps); sh = b_row - mean * sc
                        nc.scalar.activation(
                            out=sc[:, h:h + 1], in_=mv[:, h, 1:2],
                            func=mybir.ActivationFunctionType.Sqrt, bias=eps_t[:],
                        )
                        nc.vector.reciprocal(out=sc[:, h:h + 1], in_=sc[:, h:h + 1])
                        nc.vector.tensor_mul(
                            sc[:, h:h + 1], sc[:, h:h + 1], wt[:, blk:blk + 1])
                        nc.vector.tensor_mul(
                            sh[:, h:h + 1], mv[:, h, 0:1], sc[:, h:h + 1])
                        nc.vector.tensor_sub(
                            sh[:, h:h + 1], bt[:, blk:blk + 1], sh[:, h:h + 1])
                        # y = sc * x + sh, in place (Identity shares an ACT
                        # table set with Sqrt, so no table-switch cost)
                        nc.scalar.activation(
                            out=xt[:, h, :], in_=xt[:, h, :],
                            func=mybir.ActivationFunctionType.Identity,
                            bias=sh[:, h:h + 1], scale=sc[:, h:h + 1],
                        )
                    st_eng.dma_start(out=y_view, in_=xt[:])

            if loop_reps:
                with tc.For_i(0, loop_reps, 1) as _it:
                    emit_body()
            else:
                emit_body()
    nc.finalize()
    return nc


_NC = None


def _get_nc() -> bass.Bass:
    global _NC
    if _NC is None:
        _NC = build_nc()
    return _NC


def _shard_inputs(x, weight, bias, style_ids):
    """Host-side prep: gather style tables per sample, split batch across cores."""
    x = np.asarray(x)
    if x.dtype != np.float32:
        x = x.astype(np.float32)
    weight = np.asarray(weight, dtype=np.float32)
    bias = np.asarray(bias, dtype=np.float32)
    sid = np.asarray(style_ids).astype(np.int64)

    w_g = weight[sid]           # [B, C]
    b_g = bias[sid]             # [B, C]
    ntiles = ROWS // P

    in_maps = []
    for m in range(N_CORES):
        xs = np.ascontiguousarray(x[m * B_PER:(m + 1) * B_PER].reshape(ROWS, L))
        # column i of the [P, ntiles] table = rows i*128..(i+1)*128 of the shard
        wg = np.ascontiguousarray(
            w_g[m * B_PER:(m + 1) * B_PER].reshape(ntiles, P).T)
        bg = np.ascontiguousarray(
            b_g[m * B_PER:(m + 1) * B_PER].reshape(ntiles, P).T)
        in_maps.append({"x": xs, "w": wg, "b": bg})
    return in_maps


def run_sharded(x, weight, bias, style_ids, **spmd_kwargs):
    """Shard, run on cores 0-7, gather. Returns (output, BassKernelResults)."""
    in_maps = _shard_inputs(x, weight, bias, style_ids)
    res = run_bass_kernel_spmd(_get_nc(), in_maps, list(range(N_CORES)), **spmd_kwargs)
    out = np.empty((B, C, L), dtype=np.float32)
    for m in range(N_CORES):
        out[m * B_PER:(m + 1) * B_PER] = res.results[m]["y"].reshape(B_PER, C, L)
    return out, res


def kernel(x, weight, bias, style_ids):
    out, _ = run_sharded(x, weight, bias, style_ids)
    return out



# revision 2
# speedup vs baseline: 2.8266x; 2.8266x over previous
"""Conditional InstanceNorm1D on 8 Trainium2 NeuronCores.

x: [32, 256, 8192] f32. Per-(b, c) instance norm over L (biased var), then a
per-sample style affine: y = x_hat * weight[style_ids[b], c] + bias[style_ids[b], c].

Sharding: pure data parallel over batch. Each core gets 4 samples ->
1024 (b, c) rows of length 8192, processed as 8 tiles of [128 partitions, 8192].
The tiny [S, C] style tables are gathered host-side into per-row scale/shift
columns so the device kernel has no indirect addressing.

The kernel is HBM-bandwidth bound (read x once, write y once), so x and y
cross HBM as fp16: the host casts x f32->fp16 and the result fp16->f32.
The rel-err budget is 2e-2; fp16 I/O costs ~1e-3. Halving the bytes puts the
DMA roofline at ~94 us/core instead of ~187 us.

Per tile the device does:
  mean/var  : 16x bn_stats (512-elem subgroups) + bn_aggr       (VectorE)
  rstd      : sqrt(var + eps) on ScalarE, reciprocal on VectorE
  fold      : sc = rstd * w_row ; sh = b_row - mean * sc        (VectorE, [128,1])
  apply     : y = Identity(sc * x + sh) in place, fp16          (ScalarE)
Loads are issued on the sync sequencer (HWDGE), stores on the scalar
sequencer (HWDGE) so load and store issue never serialize on one queue.
"""

import numpy as np

import concourse.bacc as bacc
import concourse.bass as bass
import concourse.tile as tile
from concourse import mybir
from concourse.bass_utils import run_bass_kernel_spmd

B, C, L, S = 32, 256, 8192, 4
N_CORES = 8
B_PER = B // N_CORES            # 4 samples per core
ROWS = B_PER * C                # 1024 (b, c) rows per core
P = 128                         # SBUF partitions
EPS = 1e-5
F32 = mybir.dt.float32
F16 = mybir.dt.float16
BN_FMAX = 512                   # bn_stats free-dim hardware limit


def build_nc(rows: int = ROWS, length: int = L, xbufs: int = 4,
             reps: int = 1, loop_reps: int = 0, rpp: int = 1,
             stats_sub: int = 0, alt_queues: bool = False) -> bass.Bass:
    """reps > 1 unrolls the whole pass inside one NEFF; loop_reps > 0 wraps
    the pass in a hardware For_i loop (benchmarking only: the
    (T(R2)-T(R1))/(R2-R1) delta cancels the ~90 ms axon dispatch cost)."""
    nblocks = rows // P
    ntiles = nblocks // rpp
    nsub = length // BN_FMAX
    # stats_sub < nsub computes stats over a prefix only — WRONG results,
    # benchmarking-only knob to probe whether VectorE binds the interior.
    nsub_used = stats_sub or nsub

    # Bacc (not plain Bass): its finalize() runs generate_event_semaphores,
    # which splits multi-sem waits — TRN2 compute instructions carry at most
    # one sync wait, and walrus rejects the program otherwise.
    nc = bacc.Bacc()
    x_d = nc.dram_tensor("x", [rows, length], F16, kind="ExternalInput")
    w_d = nc.dram_tensor("w", [P, nblocks], F32, kind="ExternalInput")
    b_d = nc.dram_tensor("b", [P, nblocks], F32, kind="ExternalInput")
    y_d = nc.dram_tensor("y", [rows, length], F16, kind="ExternalOutput")

    with tile.TileContext(nc) as tc:
        with (
            tc.tile_pool(name="xp", bufs=xbufs) as xp,
            tc.tile_pool(name="consts", bufs=1) as consts,
            tc.tile_pool(name="stats", bufs=ntiles) as stats,
        ):
            wt_in = consts.tile([P, nblocks], F32)
            bt_in = consts.tile([P, nblocks], F32)
            nc.sync.dma_start(out=wt_in[:], in_=w_d[:])
            nc.sync.dma_start(out=bt_in[:], in_=b_d[:])
            # bounce through a DVE copy: walrus rejects TensorTensor
            # instructions that need a DMA-sem wait (1 wait slot), so make
            # the copy absorb the DMA wait and feed DVE-produced tiles to
            # the per-tile TT ops.
            wt = consts.tile([P, nblocks], F32)
            bt = consts.tile([P, nblocks], F32)
            nc.vector.tensor_copy(wt[:], wt_in[:])
            nc.vector.tensor_copy(bt[:], bt_in[:])
            eps_t = consts.tile([P, 1], F32)
            nc.vector.memset(eps_t[:], EPS)

            def emit_body():
                for i in range(ntiles * reps):
                    i = i % ntiles
                    rows0 = i * P * rpp
                    x_view = x_d[rows0:rows0 + P * rpp, :].rearrange(
                        "(k p) l -> p k l", p=P)
                    y_view = y_d[rows0:rows0 + P * rpp, :].rearrange(
                        "(k p) l -> p k l", p=P)
                    xt = xp.tile([P, rpp, length], F16)
                    # alt_queues: alternate tiles between the two HWDGE
                    # rings (SP / ACT sequencers), stores on the other ring
                    ld = nc.scalar if (alt_queues and i % 2) else nc.sync
                    st_eng = nc.sync if (alt_queues and i % 2) else nc.scalar
                    ld.dma_start(out=xt[:], in_=x_view)

                    st = stats.tile([P, rpp, nsub_used, 6], F32)
                    mv = stats.tile([P, rpp, 2], F32)
                    sc = stats.tile([P, rpp], F32)
                    sh = stats.tile([P, rpp], F32)
                    for h in range(rpp):
                        xr = xt[:, h, :].rearrange("p (n f) -> p n f", f=BN_FMAX)
                        for j in range(nsub_used):
                            nc.vector.bn_stats(out=st[:, h, j, :], in_=xr[:, j, :])
                        nc.vector.bn_aggr(out=mv[:, h, :], in_=st[:, h])
                        blk = i * rpp + h
                        # sc = w_row / sqrt(var + eps); sh = b_row - mean * sc
                        nc.scalar.activation(
                            out=sc[:, h:h + 1], in_=mv[:, h, 1:2],
                            func=mybir.ActivationFunctionType.Sqrt, bias=eps_t[:],
                        )
                        nc.vector.reciprocal(out=sc[:, h:h + 1], in_=sc[:, h:h + 1])
                        nc.vector.tensor_mul(
                            sc[:, h:h + 1], sc[:, h:h + 1], wt[:, blk:blk + 1])
                        nc.vector.tensor_mul(
                            sh[:, h:h + 1], mv[:, h, 0:1], sc[:, h:h + 1])
                        nc.vector.tensor_sub(
                            sh[:, h:h + 1], bt[:, blk:blk + 1], sh[:, h:h + 1])
                        # y = sc * x + sh, in place (Identity shares an ACT
                        # table set with Sqrt, so no table-switch cost)
                        nc.scalar.activation(
                            out=xt[:, h, :], in_=xt[:, h, :],
                            func=mybir.ActivationFunctionType.Identity,
                            bias=sh[:, h:h + 1], scale=sc[:, h:h + 1],
                        )
                    st_eng.dma_start(out=y_view, in_=xt[:])

            if loop_reps:
                with tc.For_i(0, loop_reps, 1) as _it:
                    emit_body()
            else:
                emit_body()
    nc.finalize()
    return nc


_NC = None


def _get_nc() -> bass.Bass:
    global _NC
    if _NC is None:
        _NC = build_nc()
    return _NC


def _shard_inputs(x, weight, bias, style_ids):
    """Host-side prep: gather style tables per sample, split batch across cores."""
    x = np.asarray(x)
    x16 = x.astype(np.float16)
    weight = np.asarray(weight, dtype=np.float32)
    bias = np.asarray(bias, dtype=np.float32)
    sid = np.asarray(style_ids).astype(np.int64)

    w_g = weight[sid]           # [B, C]
    b_g = bias[sid]             # [B, C]
    ntiles = ROWS // P

    in_maps = []
    for m in range(N_CORES):
        xs = np.ascontiguousarray(x16[m * B_PER:(m + 1) * B_PER].reshape(ROWS, L))
        # column i of the [P, ntiles] table = rows i*128..(i+1)*128 of the shard
        wg = np.ascontiguousarray(
            w_g[m * B_PER:(m + 1) * B_PER].reshape(ntiles, P).T)
        bg = np.ascontiguousarray(
            b_g[m * B_PER:(m + 1) * B_PER].reshape(ntiles, P).T)
        in_maps.append({"x": xs, "w": wg, "b": bg})
    return in_maps


def run_sharded(x, weight, bias, style_ids, **spmd_kwargs):
    """Shard, run on cores 0-7, gather. Returns (output, BassKernelResults)."""
    in_maps = _shard_inputs(x, weight, bias, style_ids)
    res = run_bass_kernel_spmd(_get_nc(), in_maps, list(range(N_CORES)), **spmd_kwargs)
    out = np.empty((B, C, L), dtype=np.float32)
    for m in range(N_CORES):
        out[m * B_PER:(m + 1) * B_PER] = (
            res.results[m]["y"].astype(np.float32).reshape(B_PER, C, L))
    return out, res


def kernel(x, weight, bias, style_ids):
    out, _ = run_sharded(x, weight, bias, style_ids)
    return out
